# revision 33
# baseline (speedup 1.0000x reference)
"""Distributed Trainium2 kernel for pre-LN multi-head self-attention.

Reference computation (n=2048, d=1024, 16 heads x 64):
    xn  = LayerNorm(x) * ln_scale + ln_bias
    qkv = xn @ w_qkv ; split -> q,k,v [16, 2048, 64]
    sim = (q @ k^T) * d**-0.5 ; attn = softmax(sim)
    out = concat_heads(attn @ v) @ w_out + b_out

Sharding: 2 heads per core (tensor parallel). Each core:
  - computes LayerNorm(x) (replicated) and xn^T via PE transposes
  - projects its 2 heads' q/k/v (ln_scale folded into weights on host,
    ln_bias folded into a per-output-column bias added at PSUM evacuation)
  - attention in transposed layout (keys on partitions) so no transposes
    are needed between the two attention matmuls; a ones-column appended
    to v yields softmax denominators for free
  - ONE AllGather per 512-row chunk carrying both heads (chunked so the
    collective chain overlaps attention compute)
  - computes a 128-column slice of the final projection (+ bias)
Host assembles the 8 [128, 2048] outT shards into the [2048, 1024] output.

v2 changes vs baseline (254us):
  - x shipped bf16 (half the input DMA, 4x DVE mode for the LN normalize)
  - exp batched per key-chunk pair (FD=2048 ACT calls) via a 4-bank PSUM
    sim buffer; attn@v interleaved same-stage (lag 2 chunks) so the last
    stage drains in ~1us
  - one merged AllGather per stage (both heads) instead of two
  - ONE rearranged gather DMA per projection instead of 16 serialized
    sync-queue DMAs
  - softmax reciprocal via reciprocal_approx_fast (~5x faster)

v4 changes vs v2:
  - sim PSUM split per (key-chunk, head): 4-deep ring of single-bank
    tiles halves the sim->exp ping-pong quantum; attention slots run at
    ~875ns (PE-limited) instead of ~1.4us
  - stage-0 sims interleaved into the A-C loop (simp coexists with the
    A-C PSUM pools; warm burst moved into mmp), so after A-C only the
    stage-0 attn@v burst + normalize precede AG0
  - 4x512 AllGather chain: the CC engine charges ~9us fixed between
    consecutive collectives, so fewer chunks win (stage width capped at
    512 by the po2 accumulator)
"""

import sys

import ml_dtypes
import numpy as np

for _p in ("/opt/trn_rl_repo", "/root/.axon_site/_ro/trn_rl_repo"):
    if _p not in sys.path:
        sys.path.append(_p)

N = 2048          # sequence length
D = 1024          # model dim
HEADS = 16
DH = 64
NCORES = 8
HL = HEADS // NCORES          # heads per core (2)
HC = HL * DH                  # head cols per core (128)
LN_EPS = 1e-6
SIM_SCALE = float(D) ** -0.5  # reference scales by input dim

P = 128
RT = N // P        # 16 row tiles
DC = D // P        # 8 dim chunks
RC_W = 512         # row-chunk width for attention/collective pipeline
NRC = N // RC_W    # 4 row chunks

MM_DT = "bf16"

# DVE-side fast-exp (Schraudolph bf16 bit trick) for these key chunks of
# every stage; offloads the ACT engine (the attention-phase bottleneck) at
# ~3% max per-element error on the affected attention weights.
DVE_EXP_KCS = (3, 7, 11, 15)
_SCH_A = 128.0 / float(np.log(2.0))          # per (sim*SIM_SCALE) logit
_SCH_B = 127.0 * 128.0 - 5.5                 # exponent bias - centering

_BUILT = None


def _build():
    """Build the SPMD Bass graph (same graph on all 8 cores)."""
    from contextlib import ExitStack

    import concourse.tile as tile
    from concourse import bacc, mybir
    from concourse.masks import make_identity

    f32 = mybir.dt.float32
    dt_mm = {"f32": f32, "f32r": mybir.dt.float32r,
             "bf16": mybir.dt.bfloat16}[MM_DT]
    AF = mybir.ActivationFunctionType

    nc = bacc.Bacc(None, num_devices=NCORES)

    x_d = nc.declare_dram_parameter("x", [N, D], dt_mm, isOutput=False)
    w4_d = nc.declare_dram_parameter("w4", [D, 4 * HC], dt_mm, isOutput=False)
    b4_d = nc.declare_dram_parameter("b4", [4 * HC], f32, isOutput=False)
    out_d = nc.declare_dram_parameter("out", [HC, N], f32, isOutput=True)

    groups = [list(range(NCORES))]
    # the CC engine charges ~9us of fixed overhead BETWEEN consecutive
    # collectives (measured: AG_{i+1} starts ~9.5us after AG_i ends even
    # with its input long ready), so fewer/bigger chunks win as long as
    # attention keeps pace.  Stage width is capped at 512 by the po2 PSUM
    # accumulator (2 halves x [128, 2*512] f32 = 4 banks).
    chunks = [(0, 512), (512, 512), (1024, 512), (1536, 512)]
    S = len(chunks)

    with ExitStack() as ctx:
        tc = ctx.enter_context(tile.TileContext(nc))

        dram = ctx.enter_context(tc.tile_pool(name="dram", bufs=1, space="DRAM"))
        ag_in = [dram.tile([P, w], dt_mm, name=f"ag_in{i}")
                 for i, (_, w) in enumerate(chunks)]
        ag_out = [dram.tile([NCORES * P, w], dt_mm, addr_space="Shared",
                            name=f"ag_out{i}") for i, (_, w) in enumerate(chunks)]
        warm_in = dram.tile([1, 64], dt_mm, name="agw_in")
        warm_out = dram.tile([NCORES, 64], dt_mm, addr_space="Shared",
                             name="agw_out")

        singles = ctx.enter_context(tc.tile_pool(name="singles", bufs=1))

        # dummy AllGather to warm the collective path (queue/ring setup)
        # while the compute phases run, so the first real AG starts promptly
        nc.gpsimd.collective_compute(
            "AllGather", mybir.AluOpType.bypass, replica_groups=groups,
            ins=[warm_in[:].opt()], outs=[warm_out[:].opt()],
        )

        # x-tile DMAs first: compute starts as soon as tile 0 lands, and the
        # weight loads (needed ~20us later) queue behind them.
        x_tiles = [singles.tile([P, D], dt_mm, name=f"x{rt}")
                   for rt in range(RT)]
        for rt in range(RT):
            nc.sync.dma_start(out=x_tiles[rt],
                              in_=x_d[rt * P:(rt + 1) * P, :])

        ident = singles.tile([P, P], dt_mm)
        make_identity(nc, ident)
        warm_rhs = singles.tile([P, RC_W], dt_mm)
        nc.vector.memset(warm_rhs, 0.0)
        eps_t = singles.tile([P, 1], f32)
        nc.vector.memset(eps_t, LN_EPS)

        # weights / biases: single fused DMA each (wq|wk|wv|wo, qb|kb|vb|bo)
        w4_sb = singles.tile([P, DC, 4, HC], dt_mm)
        nc.sync.dma_start(
            out=w4_sb,
            in_=w4_d[:, :].rearrange("(c p) (g m) -> p c g m", p=P, g=4),
        )
        wq_sb, wk_sb, wv_sb, wo_sb = (w4_sb[:, :, g, :] for g in range(4))
        b4_t = singles.tile([P, 4], f32)
        nc.sync.dma_start(out=b4_t, in_=b4_d[:].rearrange("(g p) -> p g", g=4))
        qb_t, kb_t, vb_t, bo_t = (b4_t[:, g:g + 1] for g in range(4))

        # long-lived activations
        qT = singles.tile([P, N], dt_mm)        # [2*64 qdims, rows]
        kT = singles.tile([P, N], dt_mm)
        v_sb = singles.tile([P, RT, HL, DH + 1], dt_mm)  # [keys, rt, h, v|1]
        attn_h = [singles.tile([DH, N], dt_mm, name=f"attn_h{h}")
                  for h in range(HL)]
        outT = singles.tile([P, N], f32)

        nc.gpsimd.memset(v_sb[:, :, :, DH:], 1.0)  # ones column

        # attention sim/exp pools are opened around BOTH phases so stage-0
        # sims can interleave into the A-C loop as each key block's kT/v
        # lands (PSUM: tp 1-2 + mmp 2 + simp 4 banks; po2 allocated only
        # after the A-C pools close).  One single-bank PSUM tile per
        # (kc, head) from a 4-deep ring halves the sim->exp ping-pong
        # quantum, so the PE stream stops stalling on the exp evacuation.
        expp = ctx.enter_context(tc.tile_pool(name="expp", bufs=1))
        simp = ctx.enter_context(tc.tile_pool(name="simp", bufs=4, space="PSUM"))
        exp_t = expp.tile([P, RT, HL, RC_W], dt_mm, tag="exp")

        def sim_exp(idx, kc):
            """Both heads' sim for one key chunk + exp evacuation."""
            r0, w = chunks[idx]
            for h in range(HL):
                ps = simp.tile([P, RC_W], f32, tag="ps",
                               name=f"ps{idx}_{kc}_{h}")
                nc.tensor.matmul(
                    ps[:, 0:w],
                    kT[h * DH:(h + 1) * DH, kc * P:(kc + 1) * P],
                    qT[h * DH:(h + 1) * DH, r0:r0 + w],
                    start=True, stop=True,
                )
                if kc in DVE_EXP_KCS:
                    # Schraudolph: bf16 bits = int16(logit*128/ln2 + B)
                    nc.vector.tensor_scalar(
                        out=exp_t[:, kc, h, 0:w].bitcast(mybir.dt.int16),
                        in0=ps[:, 0:w],
                        scalar1=SIM_SCALE * _SCH_A, scalar2=_SCH_B,
                        op0=mybir.AluOpType.mult, op1=mybir.AluOpType.add,
                    )
                else:
                    nc.scalar.activation(
                        out=exp_t[:, kc, h, 0:w], in_=ps[:, 0:w],
                        func=AF.Exp, scale=SIM_SCALE,
                    )

        # ---- stages A-C: LayerNorm -> xn^T -> q/k/v, fused per 4-row group --
        with (
            tc.tile_pool(name="xp", bufs=3) as xp,
            tc.tile_pool(name="stat", bufs=4) as statp,
            tc.tile_pool(name="tp", bufs=2, space="PSUM") as tp,
            tc.tile_pool(name="mmp", bufs=2, space="PSUM") as mmp,
            tc.tile_pool(name="xnTp", bufs=1) as xnTp,
        ):
            xnT = xnTp.tile([P, DC, N], dt_mm)   # [dim%128, dimchunk, rows]
            vT = xnTp.tile([P, N], dt_mm)

            # short dependency-free matmul burst before the first real PE work
            warm_ps = mmp.tile([P, 512], f32, tag="pm")
            for _ in range(10):
                nc.tensor.matmul(warm_ps, ident, warm_rhs,
                                 start=True, stop=True)

            for g4 in range(RT // 4):
                for rt in range(g4 * 4, g4 * 4 + 4):
                    x_t = x_tiles[rt]
                    stats = statp.tile([P, 2, 6], f32, tag="st")
                    for sg in range(2):
                        nc.vector.bn_stats(
                            out=stats[:, sg, :],
                            in_=x_t[:, sg * 512:(sg + 1) * 512],
                        )
                    mv = statp.tile([P, 2], f32, tag="mv")
                    nc.vector.bn_aggr(out=mv, in_=stats)
                    rstd = statp.tile([P, 1], f32, tag="rstd")
                    nc.scalar.activation(
                        out=rstd, in_=mv[:, 1:2], func=AF.Sqrt,
                        bias=eps_t, scale=1.0,
                    )
                    nc.vector.reciprocal(out=rstd, in_=rstd)
                    xh_t = xp.tile([P, D], dt_mm, tag="xh")
                    nc.vector.tensor_scalar(
                        out=xh_t, in0=x_t,
                        scalar1=mv[:, 0:1], scalar2=rstd,
                        op0=mybir.AluOpType.subtract, op1=mybir.AluOpType.mult,
                    )
                    # transpose row tile into xnT: 8 [128,128] PE transposes,
                    # 4 per PSUM bank pair; evacuation 1/3 DVE, 2/3 ACT
                    for g in range(2):
                        pt = tp.tile([P, 512], dt_mm, tag="pt")
                        with nc.allow_low_precision(reason="transpose copy"):
                            for j in range(4):
                                dc = g * 4 + j
                                nc.tensor.transpose(
                                    pt[:, j * P:(j + 1) * P],
                                    xh_t[:, dc * P:(dc + 1) * P],
                                    ident,
                                )
                        dst = xnT[:, g * 4:(g + 1) * 4, rt * P:(rt + 1) * P]
                        tsrc = pt[:].rearrange("p (j q) -> p j q", j=4)
                        if (2 * rt + g) % 3 == 0:
                            nc.vector.tensor_copy(out=dst, in_=tsrc)
                        else:
                            nc.scalar.copy(out=dst, in_=tsrc)
                    # the first two tiles' transposes gate on their LayerNorm
                    # chain (~10us of PE idle early); dependency-free warm
                    # matmuls fill the wait and keep the p-state ramp alive
                    # so the 32us transpose stream runs at full clock
                    if rt < 2:
                        for _ in range(8):
                            nc.tensor.matmul(warm_ps, ident, warm_rhs,
                                             start=True, stop=True)

                # q/k/v projections for this 512-row block
                nt = g4
                for w_sb, b_t, dst in (
                    (wq_sb, qb_t, qT), (wk_sb, kb_t, kT), (wv_sb, vb_t, vT)
                ):
                    pm = mmp.tile([P, 512], f32, tag="pm")
                    for kc in range(DC):
                        nc.tensor.matmul(
                            pm,
                            w_sb[:, kc, :],
                            xnT[:, kc, nt * 512:(nt + 1) * 512],
                            start=(kc == 0), stop=(kc == DC - 1),
                        )
                    nc.scalar.activation(
                        out=dst[:, nt * 512:(nt + 1) * 512], in_=pm,
                        func=AF.Identity, bias=b_t, scale=1.0,
                    )
                # v^T -> v (row-major with ones column) for this block
                for rt in range(g4 * 4, g4 * 4 + 4):
                    pt = tp.tile([P, 512], dt_mm, tag="pt")
                    with nc.allow_low_precision(reason="transpose copy"):
                        nc.tensor.transpose(
                            pt[:, :P], vT[:, rt * P:(rt + 1) * P], ident
                        )
                    nc.vector.tensor_copy(
                        out=v_sb[:, rt, :, 0:DH],
                        in_=pt[:, :P].rearrange("p (h d) -> p h d", h=HL),
                    )
                # stage-0 sims for this block's key chunks: kT/qT for them
                # are now final, and these fill PE idle in the A-C pipeline
                # so AG0 fires right after an attn@v burst once A-C ends
                for kc in range(g4 * 4, g4 * 4 + 4):
                    sim_exp(0, kc)

        # ---- stage D: attention, per-stage pipeline -------------------------
        # Per stage (512 rows): sim for both heads packs into disjoint PE row
        # groups into a 2-buffer PSUM pool (sim of kc+1 overlaps exp of kc on
        # ACT); attn@v consumes exp_t same-stage with a 2-chunk lag into a
        # double-buffered accumulator (po2 halves alternate per stage) so the
        # next stage's attn@v never waits on the previous stage's normalize.
        # The v stationary carries a leading ones column, so the softmax
        # denominator lands on PSUM partition 0 where the fast custom-DVE
        # reciprocal and the GpSimd partition broadcast operate. Each stage
        # ships both heads in ONE AllGather; its projection is deferred 2
        # stages (accumulating into the idle po2 half) so the AG completes
        # before the gather DMA hits the sync queue.
        with (
            tc.tile_pool(name="rsum", bufs=6) as rsump,
            tc.tile_pool(name="op", bufs=1, space="PSUM") as op,
            tc.tile_pool(name="agp", bufs=2) as agp,
        ):
            po2 = op.tile([P, 2, HL * RC_W], f32, tag="po")      # 4 banks

            def av_pair(idx, kc):
                """attn@v for key chunk kc, both heads (alternating banks)."""
                r0, w = chunks[idx]
                for h in range(HL):
                    nc.tensor.matmul(
                        po2[0:DH + 1, idx % 2, h * RC_W:h * RC_W + w],
                        v_sb[:, kc, h, :],
                        exp_t[:, kc, h, 0:w],
                        start=(kc == 0), stop=(kc == RT - 1),
                    )

            def norm_tail(idx):
                """Normalize by softmax denominators, ship to the AG buffer."""
                r0, w = chunks[idx]
                dcs, rss, rbs = [], [], []
                for h in range(HL):
                    # denominator row: PSUM p64 -> SBUF p64 (DVE, same lane),
                    # then SBUF p64 -> SBUF p0 (gpsimd DMA, off the sync
                    # queue so projection gathers can't head-of-line block it)
                    d64 = rsump.tile([P, RC_W], f32, tag="d64",
                                     name=f"d64{idx}_{h}")
                    nc.vector.tensor_copy(
                        out=d64[DH:DH + 1, 0:w],
                        in_=po2[DH:DH + 1, idx % 2, h * RC_W:h * RC_W + w],
                    )
                    dc = rsump.tile([1, RC_W], f32, tag="dc",
                                    name=f"dc{idx}_{h}")
                    nc.gpsimd.dma_start(
                        out=dc[0:1, 0:w], in_=d64[DH:DH + 1, 0:w],
                    )
                    dcs.append(dc)
                for h in range(HL):
                    rs = rsump.tile([1, RC_W], f32, tag="rs",
                                    name=f"rs{idx}_{h}")
                    nc.vector.reciprocal_approx_fast(
                        out=rs[0:1, 0:w], in_=dcs[h][0:1, 0:w]
                    )
                    rss.append(rs)
                for h in range(HL):
                    rb = rsump.tile([DH, RC_W], f32, tag="rb",
                                    name=f"rb{idx}_{h}")
                    nc.gpsimd.partition_broadcast(
                        out_ap=rb[:, 0:w], in_ap=rss[h][0:1, 0:w],
                    )
                    rbs.append(rb)
                for h in range(HL):
                    with nc.allow_low_precision(reason="attn bf16 wire"):
                        nc.vector.tensor_mul(
                            out=attn_h[h][:, r0:r0 + w],
                            in0=po2[0:DH, idx % 2, h * RC_W:h * RC_W + w],
                            in1=rbs[h][:, 0:w],
                        )
                    nc.sync.dma_start(
                        out=ag_in[idx][h * DH:(h + 1) * DH, :],
                        in_=attn_h[h][:, r0:r0 + w],
                    )
                nc.gpsimd.collective_compute(
                    "AllGather",
                    mybir.AluOpType.bypass,
                    replica_groups=groups,
                    ins=[ag_in[idx][:].opt()],
                    outs=[ag_out[idx][:].opt()],
                )

            def proj(idx, half):
                """outT slice for this row chunk from the gathered heads."""
                r0, w = chunks[idx]
                agt = agp.tile([P, DC, RC_W], dt_mm, tag="agt",
                               name=f"agt{idx}")
                # per-dim-chunk gather DMAs so the first matmuls start while
                # the rest of the gathered block is still in flight
                src = ag_out[idx][:, :].rearrange("(c p) w -> p c w", p=P)
                for kc in range(DC):
                    nc.sync.dma_start(
                        out=agt[:, kc, 0:w], in_=src[:, kc, :],
                    )
                pf = po2[:, half, 0:RC_W]
                for kc in range(DC):
                    nc.tensor.matmul(
                        pf[:, 0:w],
                        wo_sb[:, kc, :],
                        agt[:, kc, 0:w],
                        start=(kc == 0), stop=(kc == DC - 1),
                    )
                # evacuate on DVE, not ACT: an ACT evac here queues ahead of
                # later exp calls and stalls the attention stream while the
                # gather DMA + matmuls complete
                nc.vector.tensor_scalar(
                    out=outT[:, r0:r0 + w], in0=pf[:, 0:w],
                    scalar1=bo_t, scalar2=None,
                    op0=mybir.AluOpType.add,
                )
                nc.sync.dma_start(
                    out=out_d[:, r0:r0 + w], in_=outT[:, r0:r0 + w]
                )

            # flat pipeline: attn@v trails sim/exp by 2 slots ACROSS stage
            # boundaries, so the PE stream never drains at a stage edge;
            # norm_tail(idx) is emitted as soon as its last attn@v is.
            # ALL projections run in the drain: inline projections queue PE
            # matmuls behind an AllGather wait and stall the attention
            # stream; the AG chain bounds the tail either way, and the
            # drained projections execute inside its gaps.
            slots = [(idx, kc) for idx in range(S) for kc in range(RT)]
            for i, (idx, kc) in enumerate(slots):
                if idx > 0:
                    sim_exp(idx, kc)
                if i >= 2:
                    pidx, pkc = slots[i - 2]
                    av_pair(pidx, pkc)
                    if pkc == RT - 1:
                        norm_tail(pidx)
            for pidx, pkc in slots[-2:]:
                av_pair(pidx, pkc)
            norm_tail(S - 1)
            for idx in range(S):
                proj(idx, idx % 2)

    if not nc.is_finalized():
        nc.finalize()
    return nc


def _get_built():
    global _BUILT
    if _BUILT is None:
        _BUILT = _build()
    return _BUILT


def _shard_inputs(x, ln_scale, ln_bias, w_qkv, w_out, b_out):
    """Host-side sharding: slice per-head weight columns, fold LN params."""
    ln_scale = np.asarray(ln_scale, np.float32)
    ln_bias = np.asarray(ln_bias, np.float32)
    w_qkv = np.asarray(w_qkv, np.float32)
    w_out = np.asarray(w_out, np.float32)
    b_out = np.asarray(b_out, np.float32)

    w_np = {"f32": np.float32, "f32r": np.float32,
            "bf16": ml_dtypes.bfloat16}[MM_DT]
    x = np.ascontiguousarray(np.asarray(x, np.float32).astype(w_np))

    in_maps = []
    for ci in range(NCORES):
        c0 = ci * HC
        ws, bs = [], []
        for off in (0, HEADS * DH, 2 * HEADS * DH):
            w = w_qkv[:, off + c0: off + c0 + HC]
            ws.append(ln_scale[:, None] * w)
            bs.append(ln_bias @ w)
        ws.append(w_out[:, c0:c0 + HC])
        bs.append(b_out[c0:c0 + HC])
        in_maps.append({
            "x": x,
            "w4": np.ascontiguousarray(
                np.concatenate(ws, axis=1).astype(w_np)),
            "b4": np.ascontiguousarray(
                np.concatenate(bs).astype(np.float32)),
        })
    return in_maps


def kernel(x, ln_scale, ln_bias, w_qkv, w_out, b_out):
    from concourse.bass_utils import run_bass_kernel_spmd

    nc = _get_built()
    in_maps = _shard_inputs(x, ln_scale, ln_bias, w_qkv, w_out, b_out)
    res = run_bass_kernel_spmd(nc, in_maps, core_ids=list(range(NCORES)))
    shards = [res.results[ci]["out"] for ci in range(NCORES)]  # [128, 2048] each
    outT = np.concatenate(shards, axis=0)  # [1024, 2048]
    return np.ascontiguousarray(outT.T)

